# revision 1
# baseline (speedup 1.0000x reference)
"""Causal self-attention (B=4, S=2048, D=1024, single head) on 8 TRN2 cores.

Sharding: core c = (batch b = c//2, key-half h = c%2). Each core computes,
for its batch, the q/k/v projections and a *partial* causal attention over
its 1024 keys (8 k-tiles of 128), chosen so both halves have identical
work profiles: for each 512-query diagonal class j, half h owns the 256
keys at physical rows [512j+256h, 512j+256h+256). Every core runs the same
program; per-core behaviour enters only through the input data: the host
permutes each core's query columns (rotate each 512-block by 256h) so its
own keys always sit at slot columns [512j, 512j+256), and ships per-core
causal masks. The device gathers its keys from the permuted xq directly.

Score trick: scores = q.k = x_q (Wq^T Wk) x_k^T, so the host precomputes
M = Wq^T @ Wk once and the device needs NO q-projection at all:
  ktilde[i, key] = M @ x_k^T       (lhsT=M^T tile, rhs=xq key-columns)
  v[key, dout]   = x_k @ Wv^T      (lhsT=xq key-columns, rhs=WvT)
  per q-block j (512 queries), kslot s < 2j+2:
    S^T[k,q] = ktilde_s^T @ xq_j ; P = exp(S^T/32) * mask_s (diag class)
    o[q,:]  += P^T @ v ;  rowsum[q] += P^T @ ones
All operands fp16 (psum accumulation fp32). Host un-permutes rows and
merges: out_b = (o_A + o_B) / (rs_A + rs_B).
"""

import numpy as np
import ml_dtypes

import concourse.bass as bass
import concourse.mybir as mybir
import concourse.tile as tile
from concourse import bacc

B, S, D = 4, 2048, 1024
N_CORES = 8
NT = D // 128  # 8 contraction tiles
f32 = mybir.dt.float32
f32r = mybir.dt.float32r
bf16 = mybir.dt.bfloat16
f16 = mybir.dt.float16
EXP_SCALE = 1.0 / 32.0  # 1/sqrt(D)
F16 = np.float16


def _emit_body(nc, tc, ctx, xq_d, mt_d, wv_d, mk_d, ones_d, o_d, rs_d):
    from contextlib import ExitStack

    persist = ctx.enter_context(tc.tile_pool(name="persist", bufs=1))
    ps512 = ctx.enter_context(tc.tile_pool(name="ps512", bufs=4, space="PSUM"))
    kt = [persist.tile([128, 1024], f16, tag=f"kt{i}", name=f"kt{i}") for i in range(NT)]
    vt = [persist.tile([128, 1024], f16, tag=f"vt{i}", name=f"vt{i}") for i in range(NT)]
    rs_t = persist.tile([128, 16], f32, tag="rs", name="rs_t")
    ones_t = persist.tile([128, 4], f16, tag="ones", name="ones_t")
    nc.sync.dma_start(out=ones_t, in_=ones_d[:, :])

    # xq stays resident the whole kernel: rhs of ktilde (key cols), lhsT of v,
    # and rhs of S^T (query blocks).
    xq_pool = ctx.enter_context(tc.tile_pool(name="xq", bufs=1))
    xq_s = [xq_pool.tile([128, 2048], f16, tag=f"xq{i}", name=f"xq{i}") for i in range(NT)]
    # key columns (first 256 of each 512-block) arrive first on the sync
    # queue so the ktilde projection can start immediately; the query-only
    # columns follow once the projections are underway.
    for half in range(2):  # chunk-0 key columns land first: first matmul
        for i in range(NT):   # group is gated on them
            src_k = xq_d[i * 128 : (i + 1) * 128, :].rearrange(
                "p (a c) -> p a c", c=512
            )[:, 2 * half : 2 * half + 2, 0:256]
            dst_k = xq_s[i].rearrange("p (a c) -> p a c", c=512)[
                :, 2 * half : 2 * half + 2, 0:256
            ]
            nc.sync.dma_start(out=dst_k, in_=src_k)

    # key slot s lives at xq columns [512*(s//2) + 128*(s%2), +128)
    def key_cols(s):
        c0 = 512 * (s // 2) + 128 * (s % 2)
        return c0, c0 + 128

    # ---- ktilde and v projections ----
    with ExitStack() as kv_scope:
        pkv = kv_scope.enter_context(tc.tile_pool(name="pkv", bufs=1))
        mt_s = [pkv.tile([128, 1024], f16, tag=f"mt{i}", name=f"mt{i}") for i in range(NT)]
        wv_s = [pkv.tile([128, 1024], f16, tag=f"wv{i}", name=f"wv{i}") for i in range(NT)]
        for i in range(NT):
            nc.scalar.dma_start(out=mt_s[i], in_=mt_d[i * 128 : (i + 1) * 128, :])
        for i in range(NT):
            nc.sync.dma_start(out=wv_s[i], in_=wv_d[i * 128 : (i + 1) * 128, :])
        for i in range(NT):
            src_q = xq_d[i * 128 : (i + 1) * 128, :].rearrange(
                "p (a c) -> p a c", c=512
            )[:, :, 256:512]
            dst_q = xq_s[i].rearrange("p (a c) -> p a c", c=512)[:, :, 256:512]
            nc.scalar.dma_start(out=dst_q, in_=src_q)

        # ktilde: out [i-tile 128, 512 keys of chunk]; keys of chunk c are xq
        # columns [512c:512c+256) and [512(c+2)?...] -> kslots 4c..4c+3 sit at
        # xq column blocks {512*2c..+256, 512*(2c+1)..+256}
        for chunk in range(2):
            for it in range(8):
                ps = ps512.tile([128, 512], f32, tag="ps512", name="kt_ps")
                for j in range(NT):
                    rhs = xq_s[j].rearrange("p (a c) -> p a c", c=512)[
                        :, 2 * chunk : 2 * chunk + 2, 0:256
                    ]
                    nc.tensor.matmul(
                        ps,
                        mt_s[j][:, it * 128 : (it + 1) * 128],
                        rhs,
                        start=(j == 0),
                        stop=(j == NT - 1),
                    )
                nc.vector.tensor_copy(
                    out=kt[it][:, chunk * 512 : (chunk + 1) * 512], in_=ps
                )
        # v: [key 128, dout 512] tiles, lhsT = xq key columns
        for s in range(8):
            c0, c1 = key_cols(s)
            for dc in range(2):
                ps = ps512.tile([128, 512], f32, tag="ps512", name="v_ps")
                for j in range(NT):
                    nc.tensor.matmul(
                        ps,
                        xq_s[j][:, c0:c1],
                        wv_s[j][:, dc * 512 : (dc + 1) * 512],
                        start=(j == 0),
                        stop=(j == NT - 1),
                    )
                nc.vector.tensor_copy(
                    out=vt[s][:, dc * 512 : (dc + 1) * 512], in_=ps
                )

    # masks prefetched so the attention phase never waits on them
    mpool = ctx.enter_context(tc.tile_pool(name="mk", bufs=1))
    mk_s = [mpool.tile([128, 512], f16, tag=f"mk{i}", name=f"mk{i}") for i in range(NT)]
    for i in range(NT):
        nc.sync.dma_start(out=mk_s[i], in_=mk_d[i, :, :])

    # ---- Attention ----
    with ExitStack() as att_scope:
        pt_pool = att_scope.enter_context(tc.tile_pool(name="pt", bufs=1))
        osb_pool = att_scope.enter_context(tc.tile_pool(name="osb", bufs=4))
        o_ps = att_scope.enter_context(tc.tile_pool(name="o_ps", bufs=2, space="PSUM"))
        o1_ps = att_scope.enter_context(tc.tile_pool(name="o1_ps", bufs=1, space="PSUM"))
        os_ps = att_scope.enter_context(tc.tile_pool(name="os_ps", bufs=1, space="PSUM"))

        for j in range(4):
            nk = 2 * j + 2  # kslots 0..nk-1 pair with q-block j
            pts = []
            for s in range(nk):
                # odd diagonal kslot: query cols [0:128) are non-causal on
                # both core halves and its t=0 PV is skipped, so compute
                # only cols [128:512)
                c0 = 128 if s == 2 * j + 1 else 0
                sp = ps512.tile([128, 512], f32, tag="ps512", name="st_sp")
                for dt in range(NT):
                    nc.tensor.matmul(
                        sp[:, c0:512],
                        kt[dt][:, s * 128 : (s + 1) * 128],
                        xq_s[dt][:, j * 512 + c0 : (j + 1) * 512],
                        start=(dt == 0),
                        stop=(dt == NT - 1),
                    )
                pt = pt_pool.tile([128, 512], f16, tag=f"pt{s}", name=f"pt_{s}")
                nc.scalar.activation(
                    out=pt[:, c0:512], in_=sp[:, c0:512],
                    func=mybir.ActivationFunctionType.Exp,
                    scale=EXP_SCALE,
                )
                if s // 2 == j:  # diagonal class: causal mask (per-core data)
                    nc.vector.tensor_mul(
                        pt[:, c0:512], pt[:, c0:512], mk_s[s][:, c0:512]
                    )
                pts.append(pt)

            for t in range(4):
                o0 = o_ps.tile([128, 512], f32, tag="o0", name="o0_ps")
                o1 = o1_ps.tile([128, 512], f32, tag="o1", name="o1_ps_t")
                osum = os_ps.tile([128, 4], f32, tag="os", name="osum_ps")
                # the odd kslot of the diagonal class has no valid keys for
                # subtile 0 on either core half; skip its PV contribution
                active = [s for s in range(nk) if not (s == 2 * j + 1 and t == 0)]
                for idx, s in enumerate(active):
                    lhs = pts[s][:, t * 128 : (t + 1) * 128]
                    st_, sp_ = (idx == 0), (idx == len(active) - 1)
                    nc.tensor.matmul(o0, lhs, vt[s][:, 0:512], start=st_, stop=sp_)
                    nc.tensor.matmul(o1, lhs, vt[s][:, 512:1024], start=st_, stop=sp_)
                    nc.tensor.matmul(osum, lhs, ones_t[:, :], start=st_, stop=sp_)
                osb = osb_pool.tile([128, 1024], f16, tag="osb", name="osb_t")
                nc.vector.tensor_copy(out=osb[:, 0:512], in_=o0)
                nc.vector.tensor_copy(out=osb[:, 512:1024], in_=o1)
                col = j * 4 + t
                nc.vector.tensor_copy(out=rs_t[:, col : col + 1], in_=osum[:, 0:1])
                q0 = j * 512 + t * 128
                nc.sync.dma_start(out=o_d[q0 : q0 + 128, :], in_=osb)
        nc.sync.dma_start(out=rs_d[:, :], in_=rs_t)


def _build_program(repeat=1):
    from contextlib import ExitStack

    nc = bacc.Bacc("TRN2", target_bir_lowering=False, debug=False, num_devices=N_CORES)
    xq_d = nc.dram_tensor("xq", [D, S], f16, kind="ExternalInput").ap()
    mt_d = nc.dram_tensor("mt", [D, D], f16, kind="ExternalInput").ap()
    wv_d = nc.dram_tensor("wv", [D, D], f16, kind="ExternalInput").ap()
    mk_d = nc.dram_tensor("mk", [8, 128, 512], f16, kind="ExternalInput").ap()
    ones_d = nc.dram_tensor("ones", [128, 4], f16, kind="ExternalInput").ap()
    o_d = nc.dram_tensor("o", [S, D], f16, kind="ExternalOutput").ap()
    rs_d = nc.dram_tensor("rs", [128, 16], f32, kind="ExternalOutput").ap()

    with tile.TileContext(nc) as tc:
        for _ in range(repeat):
            with ExitStack() as ctx:
                _emit_body(nc, tc, ctx, xq_d, mt_d, wv_d, mk_d, ones_d, o_d, rs_d)
    nc.compile()
    return nc


# slot->phys query permutation per key-half (rotate each 512-block by 256h)
def _perm(h):
    q = np.arange(S)
    blk, i = q // 512, q % 512
    return blk * 512 + (i + 256 * h) % 512


def _masks_for_half(h):
    """mk[s][ki, qi'] = 1 iff phys_key <= phys_query, in slot coords."""
    mk = np.zeros((8, 128, 512), np.float32)
    ki = np.arange(128)[:, None]
    qi = np.arange(512)[None, :]
    phys_q = (qi + 256 * h) % 512  # within-block physical query index
    for s in range(8):
        e = s % 2
        phys_k = 256 * h + 128 * e + ki
        mk[s] = (phys_k <= phys_q).astype(np.float32)
    return mk


def make_in_maps(x, Wq, Wk, Wv):
    Wq = np.asarray(Wq, dtype=np.float32)
    Wk = np.asarray(Wk, dtype=np.float32)
    # scores = x_q (Wq^T Wk) x_k^T; device lhsT needs M^T = Wk^T Wq
    mt = np.ascontiguousarray(Wk.T @ Wq).astype(F16)
    wvT = np.ascontiguousarray(np.asarray(Wv).T).astype(F16)
    masks = [_masks_for_half(0).astype(F16), _masks_for_half(1).astype(F16)]
    perms = [_perm(0), _perm(1)]
    ones = np.ones((128, 4), F16)
    in_maps = []
    for c in range(N_CORES):
        b, h = c // 2, c % 2
        xbT = np.asarray(x[b], dtype=np.float32).T  # [din, queries]
        in_maps.append(
            {
                "xq": np.ascontiguousarray(xbT[:, perms[h]]).astype(F16),
                "mt": mt,
                "wv": wvT,
                "mk": masks[h],
                "ones": ones,
            }
        )
    return in_maps


def merge_outputs(results):
    perms = [_perm(0), _perm(1)]
    out = np.empty((B, S, D), np.float32)
    for b in range(B):
        o_sum = np.zeros((S, D), np.float32)
        r_sum = np.zeros(S, np.float32)
        for h in range(2):
            r = results[2 * b + h]
            o_slot = r["o"].astype(np.float32)
            rs_slot = r["rs"].T.reshape(S).astype(np.float32)  # slot q=128*(4j+t)+r
            if h == 0:  # identity permutation
                o_sum += o_slot
                r_sum += rs_slot
            else:
                p = perms[h]
                o_sum[p] += o_slot
                r_sum[p] += rs_slot
        out[b] = o_sum / r_sum[:, None]
    return out


# ---------------- runner (once-jitted PJRT path) ----------------

_RUNNERS = {}


def _make_runner(nc):
    import jax
    from jax.experimental.shard_map import shard_map
    from jax.sharding import Mesh, PartitionSpec

    from concourse import bass2jax

    bass2jax.install_neuronx_cc_hook()
    assert nc.dbg_addr is None
    partition_name = nc.partition_id_tensor.name if nc.partition_id_tensor else None

    in_names, out_names, out_avals, zero_outs = [], [], [], []
    for alloc in nc.m.functions[0].allocations:
        if not isinstance(alloc, mybir.MemoryLocationSet):
            continue
        name = alloc.memorylocations[0].name
        if alloc.kind == "ExternalInput":
            if name != partition_name:
                in_names.append(name)
        elif alloc.kind == "ExternalOutput":
            shape = tuple(alloc.tensor_shape)
            dtype = mybir.dt.np(alloc.dtype)
            out_names.append(name)
            out_avals.append(jax.core.ShapedArray(shape, dtype))
            zero_outs.append(np.zeros(shape, dtype))
    n_params = len(in_names)
    n_outs = len(out_avals)
    all_names = in_names + out_names
    if partition_name is not None:
        all_names = all_names + [partition_name]

    def _body(*args):
        operands = list(args)
        if partition_name is not None:
            operands.append(bass2jax.partition_id_tensor())
        outs = bass2jax._bass_exec_p.bind(
            *operands,
            out_avals=tuple(out_avals),
            in_names=tuple(all_names),
            out_names=tuple(out_names),
            lowering_input_output_aliases=(),
            sim_require_finite=True,
            sim_require_nnan=True,
            nc=nc,
        )
        return tuple(outs)

    devices = jax.devices()[:N_CORES]
    mesh = Mesh(np.asarray(devices), ("core",))
    sharded = jax.jit(
        shard_map(
            _body,
            mesh=mesh,
            in_specs=(PartitionSpec("core"),) * (n_params + n_outs),
            out_specs=(PartitionSpec("core"),) * n_outs,
            check_rep=False,
        ),
        keep_unused=True,
    )

    state = {"key": None, "dev_in": None}

    def run(in_maps):
        per_core = [[np.asarray(m[name]) for name in in_names] for m in in_maps]
        import hashlib

        hsh = hashlib.blake2b(digest_size=16)
        for core in per_core:
            for arr in core:
                hsh.update(np.ascontiguousarray(arr).view(np.uint8).data)
        key = hsh.hexdigest()
        if state["key"] != key:
            concat_in = [
                np.concatenate([per_core[c][i] for c in range(N_CORES)], axis=0)
                for i in range(n_params)
            ]
            state["dev_in"] = [jax.device_put(a) for a in concat_in]
            state["key"] = key
        if state.get("dev_zeros") is None:
            state["dev_zeros"] = [
                jax.device_put(np.zeros((N_CORES * z.shape[0], *z.shape[1:]), z.dtype))
                for z in zero_outs
            ]
        out_arrs = sharded(*state["dev_in"], *state["dev_zeros"])
        return [
            {
                name: np.asarray(out_arrs[i]).reshape(N_CORES, *out_avals[i].shape)[c]
                for i, name in enumerate(out_names)
            }
            for c in range(N_CORES)
        ]

    return run


def get_runner(repeat=1):
    if repeat not in _RUNNERS:
        nc = _build_program(repeat)
        _RUNNERS[repeat] = _make_runner(nc)
    return _RUNNERS[repeat]


def kernel(x, Wq, Wk, Wv):
    run = get_runner()
    results = run(make_in_maps(x, Wq, Wk, Wv))
    return merge_outputs(results)



# revision 34
# speedup vs baseline: 2.1314x; 2.1314x over previous
"""Causal self-attention (B=4, S=2048, D=1024, single head) on 8 TRN2 cores.

Sharding: core c = (batch b = c//2, key-half h = c%2); each core owns 1024
keys of its batch (phys rows [512j+256h, 512j+256h+256) per 512-block) and
computes a partial causal attention over them; host merges the halves:
out = (o_A + o_B) / (rs_A + rs_B). The host rotates each 512-query-block by
256h so every core runs the identical program (per-core behaviour enters
via data only): in slot coords, core keys always sit at block cols [0:256).

Score trick: scores = x_q (Wq^T Wk) x_k^T, so the device needs no
q-projection: ktilde = M x_k with lhsT = M^T pre-scaled by 32 (dodges fp8
subnormals; the exp scale absorbs it).

All heavy matmuls are fp8e4m3 DoubleRow (2 contraction subtiles/matmul,
0.5 PE cycles/row); SBUF operands use pair layout [128, 2, cols].

Causal masking costs no vector work: each diagonal S^T accumulation group
gets one extra fp8 DoubleRow matmul (mbA/mbB, a 128-row triangle + block
construction) that adds -240*240*#violations to masked scores; exp then
underflows them to exact 0.

fp8 numeric repairs (early low-key-count queries dominate the rel error):
- v for key-slot 0 is computed on the host in f32 and shipped both as fp16
  (vt16) and fp8 (vt[0] lane 0).
- block-0 queries x slot-0 keys take an fp16 PV path (pt16/vt16); the fp8
  P lane for slot 0 at j=0 is zeroed so the DoubleRow pair skips it.
"""

import numpy as np
import ml_dtypes

import concourse.bass as bass
import concourse.mybir as mybir
import concourse.tile as tile
from concourse import bacc

B, S, D = 4, 2048, 1024
N_CORES = 8
NT = D // 128   # 8 contraction tiles
NP = NT // 2    # 4 DoubleRow pairs
f32 = mybir.dt.float32
f16 = mybir.dt.float16
f8 = mybir.dt.float8e4
F16 = np.float16
F8 = ml_dtypes.float8_e4m3
M_SCALE = 32.0
EXP_SCALE = 1.0 / (32.0 * M_SCALE)
DR = mybir.MatmulPerfMode.DoubleRow
Exp = mybir.ActivationFunctionType.Exp
Copy = mybir.ActivationFunctionType.Copy


def _q8(a):
    return np.asarray(a, np.float32).astype(F8)


DEBUG_DUMP = False


def _emit_body(nc, tc, ctx, t_in, o_d, rs_d, dbg=None):
    from contextlib import ExitStack

    in_d, mba_d, mbb_d, v016_d, v08_d = t_in

    persist = ctx.enter_context(tc.tile_pool(name="persist", bufs=1))
    kt = [persist.tile([128, 2048], f8, tag=f"kt{u}", name=f"kt{u}")
          .rearrange("p (l c) -> p l c", c=1024) for u in range(NP)]
    vt = [persist.tile([128, 2048], f8, tag=f"vt{u}", name=f"vt{u}")
          .rearrange("p (l c) -> p l c", c=1024) for u in range(NP)]
    vt16 = persist.tile([128, 1024], f16, tag="vt16", name="vt16")
    rs_t = persist.tile([128, 16], f32, tag="rs", name="rs_t")
    ones8 = persist.tile([128, 8], f8, tag="ones8", name="ones8")
    ones16 = persist.tile([128, 4], f16, tag="ones16", name="ones16")
    mba = persist.tile([128, 512], f8, tag="mba", name="mba") \
        .rearrange("p (e l k) -> p e l k", e=2, l=2)
    mbb = persist.tile([128, 2048], f8, tag="mbb", name="mbb") \
        .rearrange("p (e l q) -> p e l q", e=2, l=2)
    nc.gpsimd.memset(ones8, 1.0)
    nc.gpsimd.memset(ones16, 1.0)

    # one input tile per din-pair: [ mt 1024 | xq slot cols 2048 | wv 1024 ];
    # mt+xq ship first (gate ktilde), wv second (gates only v)
    xq_pool = ctx.enter_context(tc.tile_pool(name="xin", bufs=1))
    xin = [xq_pool.tile([128, 8192], f8, tag=f"xin{u}", name=f"xin{u}")
           .rearrange("p (l c) -> p l c", c=4096) for u in range(NP)]
    XQ0 = 1024  # xq column base within xin
    for u in range(NP):
        for jj in range(2):
            r0 = (2 * u + jj) * 128
            q_ = nc.sync if jj == 0 else nc.scalar
            q_.dma_start(out=xin[u][:, jj, :], in_=in_d[r0:r0 + 128, :])
    nc.sync.dma_start(out=mba.rearrange("p e l k -> p (e l k)"), in_=mba_d[:, :])
    nc.sync.dma_start(out=mbb.rearrange("p e l q -> p (e l q)"), in_=mbb_d[:, :])
    nc.scalar.dma_start(out=vt16, in_=v016_d[:, :])
    nc.scalar.dma_start(out=vt[0][:, 0, :], in_=v08_d[:, :])

    with ExitStack() as kv_scope:
        pps = kv_scope.enter_context(tc.tile_pool(name="pps", bufs=3, space="PSUM"))

        # ktilde[it, key] = M x_k (keys of block a at cols [512a:512a+256))
        for it in range(NT):
            ps = pps.tile([128, 1024], f32, tag="pps", name="kt_ps")
            for a in range(4):
                for u in range(NP):
                    nc.tensor.matmul(
                        ps[:, a * 256:(a + 1) * 256],
                        xin[u][:, :, it * 128:(it + 1) * 128],
                        xin[u][:, :, XQ0 + a * 512:XQ0 + a * 512 + 256],
                        start=(u == 0), stop=(u == NP - 1), perf_mode=DR)
            if it % 2 == 0:     # GPSIMD cannot read PSUM: DVE/Act only
                nc.vector.tensor_copy(out=kt[it // 2][:, it % 2, :], in_=ps[:, :])
            else:
                nc.scalar.activation(out=kt[it // 2][:, it % 2, :], in_=ps[:, :],
                                     func=Copy)

        # v[key, dout] = x_k Wv^T; slot 0 ships from host
        for s in range(1, 8):
            c0 = 512 * (s // 2) + 128 * (s % 2)
            ps = pps.tile([128, 1024], f32, tag="pps", name="v_ps")
            for dc in range(2):
                for u in range(NP):
                    nc.tensor.matmul(
                        ps[:, dc * 512:(dc + 1) * 512],
                        xin[u][:, :, XQ0 + c0:XQ0 + c0 + 128],
                        xin[u][:, :, 3072 + dc * 512:3072 + (dc + 1) * 512],
                        start=(u == 0), stop=(u == NP - 1), perf_mode=DR)
            if s % 2 == 0:
                nc.vector.tensor_copy(out=vt[s // 2][:, s % 2, :], in_=ps[:, :])
            else:
                nc.scalar.activation(out=vt[s // 2][:, s % 2, :], in_=ps[:, :],
                                     func=Copy)

    if dbg is not None:
        kt_dbg, vt_dbg, pt_dbg = dbg
        for u in range(NP):
            nc.sync.dma_start(out=kt_dbg[u, :, :],
                              in_=kt[u].rearrange("p l c -> p (l c)"))
            nc.sync.dma_start(out=vt_dbg[u, :, :],
                              in_=vt[u].rearrange("p l c -> p (l c)"))

    # ---- attention ----
    with ExitStack() as att_scope:
        pt_pool = att_scope.enter_context(tc.tile_pool(name="pt", bufs=2))
        osb_pool = att_scope.enter_context(tc.tile_pool(name="osb", bufs=2))
        st_ps = att_scope.enter_context(tc.tile_pool(name="st_ps", bufs=3, space="PSUM"))
        o_ps = att_scope.enter_context(tc.tile_pool(name="o_ps", bufs=2, space="PSUM"))
        os_ps = att_scope.enter_context(tc.tile_pool(name="os_ps", bufs=1, space="PSUM"))

        ptsj = {}
        pt16_h = [None]

        def emit_st(j):
            pts = [pt_pool.tile([128, 1024], f8, tag=f"pt{u}", name=f"pt{u}_{j}")
                   .rearrange("p (l c) -> p l c", c=512) for u in range(j + 1)]
            ptsj[j] = pts
            if j == 0:
                pt16_h[0] = pt_pool.tile([128, 512], f16, tag="pt16", name="pt16_0")
                nc.gpsimd.memset(pts[0][:, 0, :], 0.0)
            for s in range(2 * j + 2):
                u, e = s // 2, s % 2
                diag = (u == j)
                sp = st_ps.tile([128, 512], f32, tag="st", name=f"st_{j}_{s}")
                for w in range(NP):
                    nc.tensor.matmul(
                        sp,
                        kt[w][:, :, s * 128:(s + 1) * 128],
                        xin[w][:, :, XQ0 + j * 512:XQ0 + (j + 1) * 512],
                        start=(w == 0), stop=(w == NP - 1 and not diag),
                        perf_mode=DR)
                if diag:                   # causal bias: -240*240*#violations
                    nc.tensor.matmul(sp, mba[:, e], mbb[:, e],
                                     start=False, stop=True, perf_mode=DR)
                out_ap = pt16_h[0] if (j == 0 and s == 0) else pts[u][:, e, :]
                nc.scalar.activation(out=out_ap, in_=sp, func=Exp, scale=EXP_SCALE)
            if dbg is not None and j == 1:
                for u in range(j + 1):
                    nc.sync.dma_start(out=dbg[2][u, :, :],
                                      in_=pts[u].rearrange("p l c -> p (l c)"))

        def emit_pv(j):
            pts = ptsj.pop(j)
            pt16 = pt16_h[0]
            osum16 = os_ps.tile([128, 16], f32, tag="os", name="osum_ps")
            for t in range(4):
                o = o_ps.tile([128, 1024], f32, tag="o", name="o_ps_t")
                osum = osum16[:, 4 * t:4 * t + 4]
                for u in range(j + 1):
                    lhs = pts[u][:, :, t * 128:(t + 1) * 128]
                    st_ = (u == 0)
                    sp_ = (u == j) and j != 0
                    nc.tensor.matmul(o[:, 0:512], lhs, vt[u][:, :, 0:512],
                                     start=st_, stop=sp_, perf_mode=DR)
                    nc.tensor.matmul(o[:, 512:1024], lhs, vt[u][:, :, 512:1024],
                                     start=st_, stop=sp_, perf_mode=DR)
                    nc.tensor.matmul(osum, lhs, ones8.rearrange("p (l c) -> p l c", c=4),
                                     start=st_, stop=sp_, perf_mode=DR)
                if j == 0:
                    lhs16 = pt16[:, t * 128:(t + 1) * 128]
                    nc.tensor.matmul(o[:, 0:512], lhs16, vt16[:, 0:512],
                                     start=False, stop=True)
                    nc.tensor.matmul(o[:, 512:1024], lhs16, vt16[:, 512:1024],
                                     start=False, stop=True)
                    nc.tensor.matmul(osum, lhs16, ones16[:, :], start=False, stop=True)
                if t % 2 == 0:
                    osb = osb_pool.tile([128, 2048], f16, tag="osb", name="osb_t")
                dst = osb[:, (t % 2) * 1024:(t % 2) * 1024 + 1024]
                # GPSIMD can't read PSUM; split halves over DVE/Act
                nc.vector.tensor_copy(out=dst[:, 0:512], in_=o[:, 0:512])
                if t % 2 == 1:
                    nc.scalar.activation(out=dst[:, 512:1024], in_=o[:, 512:1024],
                                         func=Copy)
                else:
                    nc.vector.tensor_copy(out=dst[:, 512:1024], in_=o[:, 512:1024])
                if j == 3:                 # last block: ship per-t (short tail)
                    q0 = j * 512 + t * 128
                    q_ = nc.sync if t % 2 == 0 else nc.scalar
                    q_.dma_start(out=o_d[q0:q0 + 128, :], in_=dst)
                elif t % 2 == 1:
                    q0 = j * 512 + (t - 1) * 128
                    dst_d = o_d[q0:q0 + 256, :].rearrange("(l p) d -> p l d", l=2)
                    q_ = nc.sync if t == 1 else nc.scalar
                    q_.dma_start(out=dst_d, in_=osb.rearrange("p (l d) -> p l d", d=1024))
            nc.vector.tensor_copy(
                out=rs_t[:, j * 4:(j + 1) * 4],
                in_=osum16.rearrange("p (t f) -> p t f", f=4)[:, :, 0:1])
            nc.sync.dma_start(out=rs_d[:, j * 4:(j + 1) * 4],
                              in_=rs_t[:, j * 4:(j + 1) * 4])

        PIPELINE = False
        if PIPELINE:
            emit_st(0)
            emit_st(1)
            emit_pv(0)
            emit_st(2)
            emit_pv(1)
            emit_st(3)
            emit_pv(2)
            emit_pv(3)
        else:
            for j in range(4):
                emit_st(j)
                emit_pv(j)


def _build_program(repeat=1):
    from contextlib import ExitStack

    nc = bacc.Bacc("TRN2", target_bir_lowering=False, debug=False, num_devices=N_CORES)
    in_d = nc.dram_tensor("xin", [D, 4096], f8, kind="ExternalInput").ap()
    mba_d = nc.dram_tensor("mba", [128, 512], f8, kind="ExternalInput").ap()
    mbb_d = nc.dram_tensor("mbb", [128, 2048], f8, kind="ExternalInput").ap()
    v016_d = nc.dram_tensor("v016", [128, 1024], f16, kind="ExternalInput").ap()
    v08_d = nc.dram_tensor("v08", [128, 1024], f8, kind="ExternalInput").ap()
    o_d = nc.dram_tensor("o", [S, D], f16, kind="ExternalOutput").ap()
    rs_d = nc.dram_tensor("rs", [128, 16], f32, kind="ExternalOutput").ap()
    t_in = (in_d, mba_d, mbb_d, v016_d, v08_d)
    dbg = None
    if DEBUG_DUMP:
        dbg = (nc.dram_tensor("ktd", [NP, 128, 2048], f8, kind="ExternalOutput").ap(),
               nc.dram_tensor("vtd", [NP, 128, 2048], f8, kind="ExternalOutput").ap(),
               nc.dram_tensor("ptd", [2, 128, 1024], f8, kind="ExternalOutput").ap())

    with tile.TileContext(nc) as tc:
        for _ in range(repeat):
            with ExitStack() as ctx:
                _emit_body(nc, tc, ctx, t_in, o_d, rs_d, dbg)
    nc.compile()
    return nc


# slot->phys query permutation per key-half (rotate each 512-block by 256h)
def _perm(h):
    q = np.arange(S)
    blk, i = q // 512, q % 512
    return blk * 512 + (i + 256 * h) % 512


def _mask_bias(h):
    """mbA [128, 2, 2, 128] / mbB [128, 2, 2, 512] fp8: for slot parity e,
    sum_{p,l} A[p,l,k] * B[p,l,q] = -240*240*#causal-violations(k, q) > 0
    exactly on masked (k, q). Violation (slot coords): with phys_k =
    256h+128e+k and phys_q = (q+256h)%512, phys_k > phys_q. Encoded as a
    128-row triangle [r<k][r >= q-128e] plus block rows: e=1 needs [q<128]
    (triangle misses k=0 there), h=1 needs [q>=256] (wrapped queries)."""
    A = np.zeros((128, 2, 2, 128), np.float32)
    Bb = np.zeros((128, 2, 2, 512), np.float32)
    p = np.arange(128)
    q = np.arange(512)
    for e in range(2):
        A[:, e, 0, :] = -240.0 * (p[:, None] < np.arange(128)[None, :])
        Bb[:, e, 0, :] = 240.0 * (q[None, :] <= (p[:, None] + 128 * e))
        A[0, e, 1, :] = -240.0
        if h == 1:
            Bb[0, e, 1, :] = 240.0 * (q >= 256)
        if e == 1:
            A[1, e, 1, :] = -240.0
            Bb[1, e, 1, :] = 240.0 * (q < 128)
    return A.reshape(128, 512).astype(F8), Bb.reshape(128, 2048).astype(F8)


def make_in_maps(x, Wq, Wk, Wv):
    Wq = np.asarray(Wq, dtype=np.float32)
    Wk = np.asarray(Wk, dtype=np.float32)
    WvT = np.ascontiguousarray(np.asarray(Wv, dtype=np.float32).T)
    mt8 = _q8(np.ascontiguousarray(Wk.T @ Wq) * M_SCALE)  # device lhsT = M^T
    wv8 = _q8(WvT)
    in_maps = []
    for c in range(N_CORES):
        b, h = c // 2, c % 2
        xbT = np.asarray(x[b], dtype=np.float32).T      # [din, phys q]
        perm = _perm(h)
        xq8 = _q8(xbT[:, perm])
        # slot-0 keys: phys rows [256h, 256h+128); v0 computed in f32
        k0 = 256 * h + np.arange(128)
        v0 = np.asarray(x[b], dtype=np.float32)[k0] @ WvT
        mba, mbb = _mask_bias(h)
        in_maps.append({
            "xin": np.ascontiguousarray(np.concatenate([mt8, xq8, wv8], axis=1)),
            "mba": mba,
            "mbb": mbb,
            "v016": v0.astype(F16),
            "v08": v0.astype(F8),
        })
    return in_maps


def merge_outputs(results):
    perms = [_perm(0), _perm(1)]
    out = np.empty((B, S, D), np.float32)
    for b in range(B):
        o_sum = np.zeros((S, D), np.float32)
        r_sum = np.zeros(S, np.float32)
        for h in range(2):
            r = results[2 * b + h]
            o_slot = r["o"].astype(np.float32)
            rs_slot = r["rs"].T.reshape(S).astype(np.float32)
            if h == 0:
                o_sum += o_slot
                r_sum += rs_slot
            else:
                p = perms[h]
                o_sum[p] += o_slot
                r_sum[p] += rs_slot
        out[b] = o_sum / r_sum[:, None]
    return out


# ---------------- runner (once-jitted PJRT path) ----------------

_RUNNERS = {}


def _make_runner(nc):
    import jax
    from jax.experimental.shard_map import shard_map
    from jax.sharding import Mesh, PartitionSpec

    from concourse import bass2jax

    bass2jax.install_neuronx_cc_hook()
    assert nc.dbg_addr is None
    partition_name = nc.partition_id_tensor.name if nc.partition_id_tensor else None

    in_names, out_names, out_avals, zero_outs = [], [], [], []
    for alloc in nc.m.functions[0].allocations:
        if not isinstance(alloc, mybir.MemoryLocationSet):
            continue
        name = alloc.memorylocations[0].name
        if alloc.kind == "ExternalInput":
            if name != partition_name:
                in_names.append(name)
        elif alloc.kind == "ExternalOutput":
            shape = tuple(alloc.tensor_shape)
            dtype = mybir.dt.np(alloc.dtype)
            out_names.append(name)
            out_avals.append(jax.core.ShapedArray(shape, dtype))
            zero_outs.append(np.zeros(shape, dtype))
    n_params = len(in_names)
    n_outs = len(out_avals)
    all_names = in_names + out_names
    if partition_name is not None:
        all_names = all_names + [partition_name]

    def _body(*args):
        operands = list(args)
        if partition_name is not None:
            operands.append(bass2jax.partition_id_tensor())
        outs = bass2jax._bass_exec_p.bind(
            *operands,
            out_avals=tuple(out_avals),
            in_names=tuple(all_names),
            out_names=tuple(out_names),
            lowering_input_output_aliases=(),
            sim_require_finite=True,
            sim_require_nnan=True,
            nc=nc,
        )
        return tuple(outs)

    devices = jax.devices()[:N_CORES]
    mesh = Mesh(np.asarray(devices), ("core",))
    sharded = jax.jit(
        shard_map(
            _body,
            mesh=mesh,
            in_specs=(PartitionSpec("core"),) * (n_params + n_outs),
            out_specs=(PartitionSpec("core"),) * n_outs,
            check_rep=False,
        ),
        keep_unused=True,
    )

    state = {"key": None, "dev_in": None}

    def run(in_maps):
        per_core = [[np.asarray(m[name]) for name in in_names] for m in in_maps]
        import hashlib

        hsh = hashlib.blake2b(digest_size=16)
        for core in per_core:
            for arr in core:
                hsh.update(np.ascontiguousarray(arr).view(np.uint8).data)
        key = hsh.hexdigest()
        if state["key"] != key:
            concat_in = [
                np.concatenate([per_core[c][i] for c in range(N_CORES)], axis=0)
                for i in range(n_params)
            ]
            state["dev_in"] = [jax.device_put(a) for a in concat_in]
            state["key"] = key
        if state.get("dev_zeros") is None:
            state["dev_zeros"] = [
                jax.device_put(np.zeros((N_CORES * z.shape[0], *z.shape[1:]), z.dtype))
                for z in zero_outs
            ]
        out_arrs = sharded(*state["dev_in"], *state["dev_zeros"])
        return [
            {
                name: np.asarray(out_arrs[i]).reshape(N_CORES, *out_avals[i].shape)[c]
                for i, name in enumerate(out_names)
            }
            for c in range(N_CORES)
        ]

    return run


def get_runner(repeat=1):
    if repeat not in _RUNNERS:
        nc = _build_program(repeat)
        _RUNNERS[repeat] = _make_runner(nc)
    return _RUNNERS[repeat]


def kernel(x, Wq, Wk, Wv):
    run = get_runner()
    results = run(make_in_maps(x, Wq, Wk, Wv))
    return merge_outputs(results)
